# revision 12
# baseline (speedup 1.0000x reference)
"""Trainium2 Bass kernel for the spiking actor-critic (LIF) network.

Math (per net, weights W1 [H, D], W2 [J, H], T=100 steps):
    cur1 = x @ W1.T + b1                      # [T, H] big GEMM (DMA bound)
    LIF layer 1 (sequential over t, elementwise over H):
        v[t] = beta*v[t-1] + cur1[t] - s[t-1];  s[t] = (v[t] > 1)
    cur2 = s1 @ W2.T + b2                     # [T, J] small GEMM
    LIF layer 2 same recurrence.
    policy = softmax([sum spk2[:, 0:10], sum spk2[:, 10:20]]), critic = mem2_final

Distribution: column-parallel (tensor parallel over D_IN) across 8 cores:
core i takes D_IN slice [i*8192, (i+1)*8192), computes partial cur1 for BOTH
nets, one AllReduce of the [128, 4*T] partial, then every core runs the tiny
sequential scan redundantly (identical SPMD graph).

Host-side prep: weights/x are transposed and sharded with numpy so every DMA
is a contiguous k-major load (no on-device transposes).

Scan trick (2 vector ops per step instead of 4): track the NEGATED membrane
vt = -v. Then with w = (-beta)*vt + cur:
    vt[t] = (vt[t-1] < -1) - w[t]        # (vt<-1) == spike of t-1
both expressible as scalar_tensor_tensor ops. Spikes for the whole history
are materialized afterwards in one vectorized op: s = (vt_hist < -1).
"""

import numpy as np

T = 100
D_IN = 65536
HID = 256
NOUT = 21  # 20 actor units + 1 critic unit (weight matrix columns)
N2P = 33   # layer-2 on-chip partition layout: actor rows 0-19, critic row 32
           # (SBUF access patterns may only start at partition 0/32/64/96)
NCORES = 8
KSH = D_IN // NCORES  # 8192 k per core
KC = KSH // 128  # 64 chunks of 128
SC = 8  # chunks per DMA slab
BETA = 0.95

_CACHE = {}


def _build_graph():
    import concourse.mybir as mybir
    import concourse.tile as tile
    from concourse import bacc

    f32 = mybir.dt.float32
    Alu = mybir.AluOpType
    Act = mybir.ActivationFunctionType

    nc = bacc.Bacc("TRN2", target_bir_lowering=False, debug=False,
                   num_devices=NCORES)

    xT = nc.dram_tensor("xT", [KSH, T], f32, kind="ExternalInput")
    aW1T = nc.dram_tensor("aW1T", [KSH, HID], f32, kind="ExternalInput")
    cW1T = nc.dram_tensor("cW1T", [KSH, HID], f32, kind="ExternalInput")
    W2T = nc.dram_tensor("W2T", [HID, NOUT], f32, kind="ExternalInput")
    b1g = nc.dram_tensor("b1g", [128, 4], f32, kind="ExternalInput")  # b1/8 by group
    b2 = nc.dram_tensor("b2", [N2P, 1], f32, kind="ExternalInput")
    sel = nc.dram_tensor("sel", [20, 2], f32, kind="ExternalInput")
    outd = nc.dram_tensor("out", [1, 3], f32, kind="ExternalOutput")

    ar_in = nc.dram_tensor("ar_in", [128, 4 * T], f32)
    ar_out = nc.dram_tensor("ar_out", [128, 4 * T], f32, addr_space="Shared")

    xT_r = xT.ap().rearrange("(c p) t -> p c t", p=128)      # [128, KC, T]
    aW1T_r = aW1T.ap().rearrange("(c p) h -> p c h", p=128)  # [128, KC, HID]
    cW1T_r = cW1T.ap().rearrange("(c p) h -> p c h", p=128)

    with tile.TileContext(nc) as tc:
        with (
            tc.tile_pool(name="wa", bufs=3) as wa_pool,
            tc.tile_pool(name="wc", bufs=3) as wc_pool,
            tc.tile_pool(name="xp", bufs=3) as x_pool,
            tc.tile_pool(name="ps", bufs=1, space="PSUM") as ps_pool,
            tc.tile_pool(name="sb", bufs=1) as sb,
            tc.tile_pool(name="scr", bufs=2) as scr,
        ):
            # ---- stage 1: layer-1 GEMM, k-sharded: cur1T partials in PSUM
            ps = [ps_pool.tile([128, T], f32, tag=f"ps{i}", name=f"ps{i}")
                  for i in range(4)]
            for k0 in range(0, KC, SC):
                xt = x_pool.tile([128, SC, T], f32, tag="xt")
                wa = wa_pool.tile([128, SC, HID], f32, tag="wa")
                wc = wc_pool.tile([128, SC, HID], f32, tag="wc")
                nc.sync.dma_start(xt[:], xT_r[:, k0:k0 + SC, :])
                nc.scalar.dma_start(wa[:], aW1T_r[:, k0:k0 + SC, :])
                nc.gpsimd.dma_start(wc[:], cW1T_r[:, k0:k0 + SC, :])
                for j in range(SC):
                    k = k0 + j
                    st = (k == 0)
                    sp = (k == KC - 1)
                    nc.tensor.matmul(ps[0][:], wa[:, j, 0:128], xt[:, j, :],
                                     start=st, stop=sp)
                    nc.tensor.matmul(ps[1][:], wa[:, j, 128:256], xt[:, j, :],
                                     start=st, stop=sp)
                    nc.tensor.matmul(ps[2][:], wc[:, j, 0:128], xt[:, j, :],
                                     start=st, stop=sp)
                    nc.tensor.matmul(ps[3][:], wc[:, j, 128:256], xt[:, j, :],
                                     start=st, stop=sp)

            # ---- stage 2: psum -> sbuf (+ b1/8), AllReduce over 8 cores
            b1sb = sb.tile([128, 4], f32)
            nc.sync.dma_start(b1sb[:], b1g.ap())
            cur_sb = sb.tile([128, 4, T], f32)
            for i in range(4):
                nc.vector.tensor_scalar(cur_sb[:, i, :], ps[i][:],
                                        b1sb[:, i:i + 1], None, Alu.add)
            nc.sync.dma_start(ar_in.ap(), cur_sb[:].rearrange("p a t -> p (a t)"))
            nc.gpsimd.collective_compute(
                "AllReduce", Alu.add,
                ins=[ar_in.ap().opt()],
                outs=[ar_out.ap().opt()],
                replica_groups=[list(range(NCORES))],
            )
            c_all = sb.tile([128, 4, T], f32)
            nc.sync.dma_start(c_all[:].rearrange("p a t -> p (a t)"), ar_out.ap())

            # ---- stage 3: layer-1 LIF scan (both nets; 512 units in [128,4])
            vh = sb.tile([128, 4, T + 1], f32)
            nc.vector.memset(vh[:, :, 0], 0.0)
            for t in range(T):
                w = scr.tile([128, 4], f32, tag="w")
                nc.vector.scalar_tensor_tensor(
                    w[:], vh[:, :, t], -BETA, c_all[:, :, t],
                    op0=Alu.mult, op1=Alu.add)
                nc.vector.scalar_tensor_tensor(
                    vh[:, :, t + 1], vh[:, :, t], -1.0, w[:],
                    op0=Alu.is_lt, op1=Alu.subtract)
            spk = sb.tile([128, 4, T], f32)
            nc.vector.tensor_scalar(spk[:], vh[:, :, 1:T + 1], -1.0, None,
                                    Alu.is_lt)

            # ---- stage 4: layer-2 GEMM  cur2T = W2comb.T @ spk1
            w2c0 = sb.tile([128, NOUT], f32)
            w2c1 = sb.tile([128, NOUT], f32)
            nc.sync.dma_start(w2c0[:], W2T.ap()[0:128, :])
            nc.sync.dma_start(w2c1[:], W2T.ap()[128:256, :])
            b2sb = sb.tile([N2P, 1], f32)
            nc.sync.dma_start(b2sb[:], b2.ap())
            ps2 = ps_pool.tile([128, T], f32, tag="psl2", name="psl2")
            nc.vector.memset(ps2[0:N2P, :], 0.0)
            nc.tensor.matmul(ps2[0:20, :], w2c0[:, 0:20], spk[:, 0, :],
                             start=True, stop=False)
            nc.tensor.matmul(ps2[0:20, :], w2c1[:, 0:20], spk[:, 1, :],
                             start=False, stop=True)
            nc.tensor.matmul(ps2[32:33, :], w2c0[:, 20:21], spk[:, 2, :],
                             start=True, stop=False)
            nc.tensor.matmul(ps2[32:33, :], w2c1[:, 20:21], spk[:, 3, :],
                             start=False, stop=True)
            c2 = sb.tile([N2P, T], f32)
            nc.vector.tensor_scalar(c2[:], ps2[0:N2P, :], b2sb[:], None,
                                    Alu.add)

            # ---- stage 5: layer-2 LIF scan (actor rows 0-19, critic row 32)
            vh2 = sb.tile([N2P, T + 1], f32)
            nc.vector.memset(vh2[:, 0:1], 0.0)
            for t in range(T):
                w2s = scr.tile([N2P, 1], f32, tag="w2s")
                nc.vector.scalar_tensor_tensor(
                    w2s[:], vh2[:, t:t + 1], -BETA, c2[:, t:t + 1],
                    op0=Alu.mult, op1=Alu.add)
                nc.vector.scalar_tensor_tensor(
                    vh2[:, t + 1:t + 2], vh2[:, t:t + 1], -1.0, w2s[:],
                    op0=Alu.is_lt, op1=Alu.subtract)

            # ---- stage 6: policy head + critic output
            s2 = sb.tile([20, T], f32)
            nc.vector.tensor_scalar(s2[:], vh2[0:20, 1:T + 1], -1.0, None,
                                    Alu.is_lt)
            u = sb.tile([20, 1], f32)
            nc.vector.tensor_reduce(u[:], s2[:], axis=mybir.AxisListType.X,
                                    op=Alu.add)
            selsb = sb.tile([20, 2], f32)
            nc.sync.dma_start(selsb[:], sel.ap())
            av = ps_pool.tile([128, 2], f32, tag="av")
            nc.tensor.matmul(av[0:1, :], u[:], selsb[:], start=True, stop=True)
            rm = sb.tile([1, 1], f32)
            nc.vector.tensor_reduce(rm[:], av[0:1, :], axis=mybir.AxisListType.X,
                                    op=Alu.max)
            avs = sb.tile([1, 2], f32)
            nc.vector.tensor_scalar(avs[:], av[0:1, :], rm[:], None, Alu.subtract)
            es = sb.tile([1, 2], f32)
            nc.scalar.activation(es[:], avs[:], Act.Exp)
            ssum = sb.tile([1, 1], f32)
            nc.vector.tensor_reduce(ssum[:], es[:], axis=mybir.AxisListType.X,
                                    op=Alu.add)
            rinv = sb.tile([1, 1], f32)
            nc.vector.reciprocal(rinv[:], ssum[:])
            pol = sb.tile([1, 2], f32)
            nc.vector.tensor_scalar(pol[:], es[:], rinv[:], None, Alu.mult)
            cmn = sb.tile([N2P, 1], f32)
            nc.vector.tensor_scalar(cmn[32:33, :], vh2[32:33, T:T + 1], -1.0,
                                    None, Alu.mult)
            nc.sync.dma_start(outd.ap()[:, 0:2], pol[:])
            nc.sync.dma_start(outd.ap()[:, 2:3], cmn[32:33, :])

    nc.compile()
    return nc


def _in_maps(poisson_spikes, aW1, ab1, aW2, ab2, cW1, cb1, cW2, cb2):
    f = np.float32
    x = np.ascontiguousarray(poisson_spikes.reshape(T, D_IN), dtype=f)
    aW1 = np.asarray(aW1, f)
    cW1 = np.asarray(cW1, f)
    W2T = np.ascontiguousarray(
        np.concatenate([np.asarray(aW2, f),
                        np.asarray(cW2, f)], axis=0).T)  # [256, 21]
    b1 = np.concatenate([np.asarray(ab1, f), np.asarray(cb1, f)])  # [512]
    b1g = np.ascontiguousarray(b1.reshape(4, 128).T) / np.float32(NCORES)
    b2v = np.zeros((N2P, 1), f)
    b2v[0:20, 0] = np.asarray(ab2, f)
    b2v[32, 0] = np.asarray(cb2, f)[0]
    selv = np.zeros((20, 2), f)
    selv[0:10, 0] = 1.0
    selv[10:20, 1] = 1.0

    maps = []
    for i in range(NCORES):
        sl = slice(i * KSH, (i + 1) * KSH)
        maps.append({
            "xT": np.ascontiguousarray(x[:, sl].T),
            "aW1T": np.ascontiguousarray(aW1[:, sl].T),
            "cW1T": np.ascontiguousarray(cW1[:, sl].T),
            "W2T": W2T,
            "b1g": np.ascontiguousarray(b1g),
            "b2": b2v,
            "sel": selv,
        })
    return maps


def kernel(**inputs):
    from concourse.bass_utils import run_bass_kernel_spmd

    if "nc" not in _CACHE:
        _CACHE["nc"] = _build_graph()
    nc = _CACHE["nc"]
    maps = _in_maps(**inputs)
    res = run_bass_kernel_spmd(nc, maps, core_ids=list(range(NCORES)))
    out = np.asarray(res.results[0]["out"], np.float32).reshape(3)
    return out[0:2].copy(), out[2:3].copy()
